# revision 6
# baseline (speedup 1.0000x reference)
"""Causal single-head attention (B=4, S=2048, D=DK=1024) on 8 trn2 NeuronCores.

Sharding: data-parallel over batch x interleaved q-blocks. Core c handles
batch b=c//2, parity p=c%2, owning the 8 q-blocks {2j+p : j in 0..7} (128 rows
each). One uniform SPMD program runs on all 8 cores; per-core differences are
carried entirely by the input data (host-side column permutation + mask tiles).

Math per core (weight-folded to skip full-context K/V projections; W_QK =
W_Q W_K^T is folded on the host):
    G^T = W_QK^T X_q^T                [d, 1024]
    S   = G X_ctx^T   (causal window, compact 2-region layout)
    A   = softmax(S/32 with -1e9 mask pre-scale)
    P   = A X_ctx
    out = P W_V       (then scatter rows back on host)

All matmul operands are fp16 (11-bit mantissa, same class as tf32): the PE
streams fp16 at 1 cycle/row like f32r, but LDWEIGHTS takes ~97 ns instead of
~224 so weight loads hide under the 512-col matmul compute (~216 ns), and
every DMA/SBUF byte count halves vs fp32.

The q-tile loop is software-pipelined one stage deep: scores+softmax of tile
j+1 are emitted between A@X and P@W_V of tile j, so each tile's softmax
latency (vector/scalar chain) hides under the previous tile's tensor work.
scores_0 runs between the two G half-passes. Inputs stream on both hardware
DGE queues (sync + scalar) in first-use order.
"""

import numpy as np

B, S, D = 4, 2048, 1024
P = 128               # partitions
NJ = 8                # q-tiles per core
NCORES = 8
MASK_FILL = -1.0e9
WARMUP = 16           # PE clock-ramp matmuls while first inputs stream in

_cache = {}


def _build_program():
    from contextlib import ExitStack
    import concourse.bass as bass
    import concourse.bacc as bacc
    import concourse.tile as tile
    import concourse.mybir as mybir
    from concourse import masks

    f32 = mybir.dt.float32
    f16 = mybir.dt.float16
    Exp = mybir.ActivationFunctionType.Exp
    Copy = mybir.ActivationFunctionType.Copy
    AX = mybir.AxisListType.X
    ts = bass.ts

    nc = bacc.Bacc("TRN2", target_bir_lowering=False, debug=False,
                   enable_asserts=False)

    xct_d = nc.dram_tensor("xct", [D, S], f16, kind="ExternalInput").ap()
    xc_d = nc.dram_tensor("xc", [S, D], f16, kind="ExternalInput").ap()
    wqk_d = nc.dram_tensor("wqk", [D, D], f16, kind="ExternalInput").ap()
    wv_d = nc.dram_tensor("wv", [D, D], f16, kind="ExternalInput").ap()
    madd_d = nc.dram_tensor("madd", [NJ * P, 2 * P], f32,
                            kind="ExternalInput").ap()
    out_d = nc.dram_tensor("out", [NJ * P, D], f32, kind="ExternalOutput").ap()

    xct_r = xct_d.rearrange("(c p) k -> c p k", p=P)    # [8, 128, 2048]
    xc_r = xc_d.rearrange("(c p) d -> c p d", p=P)      # [16, 128, 1024]
    wqk_r = wqk_d.rearrange("(c p) n -> c p n", p=P)
    wv_r = wv_d.rearrange("(c p) n -> c p n", p=P)
    madd_r = madd_d.rearrange("(j p) m -> p j m", p=P)  # [128, 8, 256]

    def alt(i):
        return nc.sync if i % 2 == 0 else nc.scalar

    with tile.TileContext(nc) as tc, ExitStack() as es:
        # ---- persistent pools -------------------------------------------
        perm = es.enter_context(tc.tile_pool(name="perm", bufs=1))
        xct_sb = perm.tile([P, 8, S], f16)         # X_ctx^T  32KB/part
        xc_sb = perm.tile([P, 16, D], f16)         # X_ctx (perm rows) 32KB/part
        gt_sb = perm.tile([P, 8, 1024], f16)       # G^T 16KB/part
        madd_sb = perm.tile([P, 8, 2 * P], f32)    # all mask tiles 8KB/part
        ident_h = perm.tile([P, P], f16)

        masks.make_identity(nc, ident_h[:])

        wv_pool = tc.alloc_tile_pool(name="wv", bufs=1, side="right")
        wv_sb = wv_pool.tile([P, 8, 1024], f16)
        # scores psum pool + softmax tiles allocated ahead of G's pools so
        # the pipelined scores_0 doesn't wait on pool release barriers
        spsp = tc.alloc_tile_pool(name="sps", bufs=2, space="PSUM")
        earlyp = tc.alloc_tile_pool(name="early", bufs=2)
        statp = tc.alloc_tile_pool(name="stats", bufs=4)

        x4 = xct_sb[:].rearrange("p d (r c) -> p d r c", r=2)

        # per-j softmax state threaded between pipeline stages
        state = {}

        def emit_scores_softmax(j):
            hw = (j + 1) * P        # per-region context width
            srow = earlyp.tile([P, 2048], f32, tag="srow")
            segs = []  # (dst_off, width, mask_region) per segment
            if hw <= 256:
                # both regions in one matmul via a 2-region moving AP
                ps = spsp.tile([P, 512], f32, tag="ps")
                for dc in range(8):
                    nc.tensor.matmul(
                        ps[:, 0:2 * hw], gt_sb[:, dc, ts(j, P)],
                        x4[:, dc, :, 0:hw],
                        start=(dc == 0), stop=(dc == 7))
                nc.vector.tensor_copy(srow[:, 0:2 * hw], ps[:, 0:2 * hw])
                segs.append((0, hw, 0))
                segs.append((hw, hw, 1))
            else:
                # equal piece split keeps every piece's compute above the
                # fp16 LDWEIGHTS time so weight loads stay hidden
                pw = hw // 2 if hw > 512 else hw
                for ri, (base_src, base_dst) in enumerate(((0, 0), (1024, hw))):
                    for off in range(0, hw, pw):
                        w = min(pw, hw - off)
                        ps = spsp.tile([P, 512], f32, tag="ps")
                        for dc in range(8):
                            nc.tensor.matmul(
                                ps[:, :w], gt_sb[:, dc, ts(j, P)],
                                xct_sb[:, dc,
                                       base_src + off:base_src + off + w],
                                start=(dc == 0), stop=(dc == 7))
                        dst = base_dst + off
                        nc.vector.tensor_copy(srow[:, dst:dst + w],
                                              ps[:, :w])
                        segs.append((dst, w, ri if off + w == hw else None))
            mxseg = statp.tile([P, 4], f32, tag="mxseg")
            for si, (dst, w, ri) in enumerate(segs):
                if ri is not None:
                    # boundary chunk of region ri sits at this segment's
                    # tail: apply the additive causal mask before the max
                    chunk = ts(j, P) if ri == 0 else ts(2 * j + 1, P)
                    nc.vector.tensor_add(srow[:, chunk], srow[:, chunk],
                                         madd_sb[:, j, ri * P:(ri + 1) * P])
                nc.vector.reduce_max(mxseg[:, si:si + 1],
                                     srow[:, dst:dst + w], axis=AX)
            nmx = statp.tile([P, 1], f32, tag="nmx")
            nc.vector.reduce_max(nmx[:], mxseg[:, :len(segs)], axis=AX,
                                 negate=True)
            nc.scalar.mul(nmx[:], nmx[:], 1.0 / 32.0)
            seseg = statp.tile([P, 4], f32, tag="seseg")
            attn = earlyp.tile([P, 2048], f16, tag="attn")
            for si, (dst, w, _) in enumerate(segs):
                nc.scalar.activation(attn[:, dst:dst + w],
                                     srow[:, dst:dst + w], Exp,
                                     bias=nmx[:], scale=1.0 / 32.0,
                                     accum_out=seseg[:, si:si + 1])
            sumexp = statp.tile([P, 1], f32, tag="se")
            nc.vector.reduce_sum(sumexp[:], seseg[:, :len(segs)], axis=AX)
            rcp = statp.tile([P, 1], f32, tag="rcp")
            nc.vector.reciprocal(rcp[:], sumexp[:])
            state[j] = (attn, rcp)

        # ---- phase G: G^T = (W_Q W_K^T)^T X_q^T -------------------------
        # HAM warm-up: dependency-free matmuls fill the PE-idle window while
        # the first input chunks stream in, so phase G starts at full clock
        warm = spsp.tile([P, 512], f32, tag="ps", name="warmup")
        for _ in range(WARMUP):
            nc.tensor.matmul(warm[:, 0:P], ident_h[:], ident_h[:])

        with tc.tile_pool(name="wqk", bufs=1) as wqkp, \
             tc.tile_pool(name="pps", bufs=6, space="PSUM") as pps:
            wqk_sb = wqkp.tile([P, 8, 1024], f16)
            # G inputs split across both DGE queues, wqk/xct interleaved in
            # consumption order
            for dc in range(8):
                alt(dc).dma_start(wqk_sb[:, dc, :], wqk_r[dc])
                alt(dc).dma_start(xct_sb[:, dc, 0:512], xct_r[dc, :, 0:512])
            for dc in range(8):
                alt(dc).dma_start(xct_sb[:, dc, 512:1024],
                                  xct_r[dc, :, 512:1024])
            # region-2 head: needed by the pipelined scores_0/1
            for dc in range(8):
                alt(dc).dma_start(xct_sb[:, dc, 1024:1280],
                                  xct_r[dc, :, 1024:1280])
            nc.scalar.dma_start(madd_sb[:], madd_r)
            nc.sync.dma_start(xc_sb[:, 0, :], xc_r[0])
            nc.scalar.dma_start(xc_sb[:, 8, :], xc_r[8])
            for dc in range(8):
                alt(dc).dma_start(wv_sb[:, dc, :], wv_r[dc])
            for i in range(1, 8):
                nc.sync.dma_start(xc_sb[:, i, :], xc_r[i])
                nc.scalar.dma_start(xc_sb[:, 8 + i, :], xc_r[8 + i])
            for dc in range(8):
                alt(dc).dma_start(xct_sb[:, dc, 1280:1536],
                                  xct_r[dc, :, 1280:1536])
            for dc in range(8):
                alt(dc).dma_start(xct_sb[:, dc, 1536:2048],
                                  xct_r[dc, :, 1536:2048])

            # qh-outer: pass 0 needs only the first xct q-half from DRAM;
            # scores_0 slots between the passes so its softmax runs under
            # pass 1.
            for qh in (0, 512):
                for g0 in range(0, 8, 2):
                    psl = {dt_: pps.tile([P, 512], f32, tag="ps",
                                         name=f"psG{dt_}{qh}")
                           for dt_ in (g0, g0 + 1)}
                    for dc in range(8):
                        for dt_ in (g0, g0 + 1):
                            nc.tensor.matmul(
                                psl[dt_][:], wqk_sb[:, dc, ts(dt_, P)],
                                xct_sb[:, dc, qh:qh + 512],
                                start=(dc == 0), stop=(dc == 7))
                    for dt_ in (g0, g0 + 1):
                        nc.scalar.copy(gt_sb[:, dt_, qh:qh + 512],
                                       psl[dt_][:])
                if qh == 0:
                    emit_scores_softmax(0)

        # ---- phase D: attention per q-tile, pipelined one stage deep ----
        with tc.tile_pool(name="work1", bufs=1) as work1, \
             tc.tile_pool(name="work2", bufs=2) as work2, \
             tc.tile_pool(name="trp", bufs=2, space="PSUM") as trp, \
             tc.tile_pool(name="ppp", bufs=2, space="PSUM") as ppp, \
             tc.tile_pool(name="ops", bufs=2, space="PSUM") as opsp:
            for j in range(NJ):
                nk = 2 * j + 2          # 128-wide k-chunks this q-tile
                attn, rcp = state.pop(j)

                attnT = work1.tile([P, 2048], f16, tag="attnT")
                for c in range(nk):
                    tp = trp.tile([P, P], f16, tag="tr")
                    nc.tensor.transpose(tp[:], attn[:, ts(c, P)], ident_h[:])
                    nc.vector.tensor_copy(attnT[:, ts(c, P)], tp[:])

                # A @ X_ctx, dv-half-outer so the half-0 copy overlaps the
                # half-1 chain
                p_sb = work2.tile([P, 1024], f16, tag="p", bufs=1)
                for dh in (0, 512):
                    pp = ppp.tile([P, 512], f32, tag="pp")
                    for c in range(nk):
                        pos = c if c <= j else 8 + (c - j - 1)
                        nc.tensor.matmul(
                            pp[:], attnT[:, ts(c, P)],
                            xc_sb[:, pos, dh:dh + 512],
                            start=(c == 0), stop=(c == nk - 1))
                    nc.vector.tensor_copy(p_sb[:, dh:dh + 512], pp[:])

                # next tile's scores+softmax hide under this tile's tail work
                if j + 1 < NJ:
                    emit_scores_softmax(j + 1)

                pt_sb = work2.tile([P, 1024], f16, tag="pt", bufs=1)
                for dc in range(8):
                    tp = trp.tile([P, P], f16, tag="tr")
                    nc.tensor.transpose(tp[:], p_sb[:, ts(dc, P)], ident_h[:])
                    nc.vector.tensor_copy(pt_sb[:, ts(dc, P)], tp[:])

                # P @ W_V, dv-half-outer; each half normalizes and streams
                # out on its own DGE queue as soon as its chain retires
                out_sb = work2.tile([P, 1024], f32, tag="out")
                for dvh, eng in ((0, nc.sync), (512, nc.scalar)):
                    op = opsp.tile([P, 512], f32, tag="op")
                    for dc in range(8):
                        nc.tensor.matmul(
                            op[:], pt_sb[:, ts(dc, P)],
                            wv_sb[:, dc, dvh:dvh + 512],
                            start=(dc == 0), stop=(dc == 7))
                    nc.scalar.activation(out_sb[:, dvh:dvh + 512], op[:],
                                         Copy, scale=rcp[:])
                    eng.dma_start(out_d[ts(j, P), dvh:dvh + 512],
                                  out_sb[:, dvh:dvh + 512])
        statp.release()
        earlyp.release()
        spsp.release()
        wv_pool.release()

    nc.compile()
    return nc


def _prep_inputs(sequence_repr, W_Q, W_K, W_V, mask):
    """Build the 8 per-core input dicts (host-side slicing/permutation)."""
    wqk = (W_Q @ W_K.T).astype(np.float16)
    wv16 = np.ascontiguousarray(W_V).astype(np.float16)
    in_maps = []
    meta = []
    for c in range(NCORES):
        b, par = divmod(c, 2)
        qblocks = [2 * j + par for j in range(NJ)]
        oblocks = [2 * j + 1 - par for j in range(NJ)]
        posblocks = qblocks + oblocks
        rows_perm = np.concatenate(
            [np.arange(g * P, (g + 1) * P) for g in posblocks])
        qrows = rows_perm[:NJ * P]
        xb = sequence_repr[b]
        xct = np.ascontiguousarray(xb.T[:, rows_perm]).astype(np.float16)
        xc = np.ascontiguousarray(xb[rows_perm]).astype(np.float16)
        madd = np.empty((NJ * P, 2 * P), np.float32)
        for j in range(NJ):
            g = 2 * j + par
            gb = 2 * j + 1 - par
            qr = slice((2 * j + par) * P, (2 * j + par) * P + P)
            madd[j * P:(j + 1) * P, 0:P] = np.where(
                mask[b, qr, g * P:(g + 1) * P], 0.0, MASK_FILL)
            madd[j * P:(j + 1) * P, P:2 * P] = np.where(
                mask[b, qr, gb * P:(gb + 1) * P], 0.0, MASK_FILL)
        in_maps.append({
            "xct": xct, "xc": xc,
            "wqk": wqk,
            "wv": wv16,
            "madd": madd,
        })
        meta.append((b, qrows))
    return in_maps, meta


def run(sequence_repr, W_Q, W_K, W_V, mask, trace=False):
    from concourse.bass_utils import run_bass_kernel_spmd

    if "nc" not in _cache:
        _cache["nc"] = _build_program()
    nc = _cache["nc"]
    in_maps, meta = _prep_inputs(
        np.asarray(sequence_repr, np.float32), np.asarray(W_Q, np.float32),
        np.asarray(W_K, np.float32), np.asarray(W_V, np.float32),
        np.asarray(mask))
    res = run_bass_kernel_spmd(nc, in_maps, core_ids=list(range(NCORES)),
                               trace=trace)
    out = np.empty((B, S, D), np.float32)
    for c in range(NCORES):
        b, qrows = meta[c]
        out[b, qrows] = res.results[c]["out"]
    return out, res


def kernel(**inputs):
    out, _ = run(**inputs)
    return out


# revision 8
# speedup vs baseline: 1.1513x; 1.1513x over previous
"""Causal single-head attention (B=4, S=2048, D=DK=1024) on 8 trn2 NeuronCores.

Sharding: data-parallel over batch x interleaved q-blocks. Core c handles
batch b=c//2, parity p=c%2, owning the 8 q-blocks {2j+p : j in 0..7} (128 rows
each). One uniform SPMD program runs on all 8 cores; per-core differences are
carried entirely by the input data (host-side column permutation + mask tiles).

Math per core (weight-folded to skip full-context K/V projections; W_QK =
W_Q W_K^T is folded on the host):
    G^T = W_QK^T X_q^T                [d, 1024]
    S   = G X_ctx^T   (causal window, compact 2-region layout)
    A   = softmax(S/32 with -1e9 mask pre-scale)
    P   = A X_ctx
    out = P W_V       (then scatter rows back on host)

All matmul operands are fp16 (11-bit mantissa, same class as tf32): the PE
streams fp16 at 1 cycle/row like f32r, but LDWEIGHTS takes ~97 ns instead of
~224 so weight loads hide under the 512-col matmul compute (~216 ns), and
every DMA/SBUF byte count halves vs fp32.

The q-tile loop is software-pipelined one stage deep: scores+softmax of tile
j+1 are emitted between A@X and P@W_V of tile j, so each tile's softmax
latency (vector/scalar chain) hides under the previous tile's tensor work.
scores_0 runs between the two G half-passes. Inputs stream on both hardware
DGE queues (sync + scalar) in first-use order.
"""

import numpy as np

B, S, D = 4, 2048, 1024
P = 128               # partitions
NJ = 8                # q-tiles per core
NCORES = 8
MASK_FILL = -1.0e9
WARMUP = 16           # PE clock-ramp matmuls while first inputs stream in

_cache = {}


def _build_program():
    from contextlib import ExitStack
    import concourse.bass as bass
    import concourse.bacc as bacc
    import concourse.tile as tile
    import concourse.mybir as mybir
    from concourse import masks

    f32 = mybir.dt.float32
    f16 = mybir.dt.float16
    Exp = mybir.ActivationFunctionType.Exp
    Copy = mybir.ActivationFunctionType.Copy
    AX = mybir.AxisListType.X
    ts = bass.ts

    nc = bacc.Bacc("TRN2", target_bir_lowering=False, debug=False,
                   enable_asserts=False)

    xct_d = nc.dram_tensor("xct", [D, S], f16, kind="ExternalInput").ap()
    xc_d = nc.dram_tensor("xc", [S, D], f16, kind="ExternalInput").ap()
    wqk_d = nc.dram_tensor("wqk", [D, D], f16, kind="ExternalInput").ap()
    wv_d = nc.dram_tensor("wv", [D, D], f16, kind="ExternalInput").ap()
    madd_d = nc.dram_tensor("madd", [NJ * P, 2 * P], f32,
                            kind="ExternalInput").ap()
    out_d = nc.dram_tensor("out", [NJ * P, D], f32, kind="ExternalOutput").ap()

    # partition-major views so one dma_start covers many 128-row chunks:
    # dma_start instructions cost ~600ns each on the issuing engine, so
    # bulk inputs go as a few large transfers on the sync engine only
    xct_p = xct_d.rearrange("(c p) k -> p c k", p=P)    # [128, 8, 2048]
    xc_p = xc_d.rearrange("(c p) d -> p c d", p=P)      # [128, 16, 1024]
    wqk_p = wqk_d.rearrange("(c p) n -> p c n", p=P)    # [128, 8, 1024]
    wv_p = wv_d.rearrange("(c p) n -> p c n", p=P)
    madd_r = madd_d.rearrange("(j p) m -> p j m", p=P)  # [128, 8, 256]

    with tile.TileContext(nc) as tc, ExitStack() as es:
        # ---- persistent pools -------------------------------------------
        perm = es.enter_context(tc.tile_pool(name="perm", bufs=1))
        xct_sb = perm.tile([P, 8, S], f16)         # X_ctx^T  32KB/part
        xc_sb = perm.tile([P, 16, D], f16)         # X_ctx (perm rows) 32KB/part
        gt_sb = perm.tile([P, 8, 1024], f16)       # G^T 16KB/part
        madd_sb = perm.tile([P, 8, 2 * P], f32)    # all mask tiles 8KB/part
        ident_h = perm.tile([P, P], f16)

        masks.make_identity(nc, ident_h[:])

        wv_pool = tc.alloc_tile_pool(name="wv", bufs=1, side="right")
        wv_sb = wv_pool.tile([P, 8, 1024], f16)
        # scores psum pool + softmax tiles allocated ahead of G's pools so
        # the pipelined scores_0 doesn't wait on pool release barriers
        spsp = tc.alloc_tile_pool(name="sps", bufs=2, space="PSUM")
        earlyp = tc.alloc_tile_pool(name="early", bufs=2)
        statp = tc.alloc_tile_pool(name="stats", bufs=4)

        x4 = xct_sb[:].rearrange("p d (r c) -> p d r c", r=2)

        # per-j softmax state threaded between pipeline stages
        state = {}

        def emit_scores_softmax(j):
            hw = (j + 1) * P        # per-region context width
            srow = earlyp.tile([P, 2048], f32, tag="srow")
            segs = []  # (dst_off, width, mask_region) per segment
            if hw <= 256:
                # both regions in one matmul via a 2-region moving AP
                ps = spsp.tile([P, 512], f32, tag="ps")
                for dc in range(8):
                    nc.tensor.matmul(
                        ps[:, 0:2 * hw], gt_sb[:, dc, ts(j, P)],
                        x4[:, dc, :, 0:hw],
                        start=(dc == 0), stop=(dc == 7))
                nc.vector.tensor_copy(srow[:, 0:2 * hw], ps[:, 0:2 * hw])
                segs.append((0, hw, 0))
                segs.append((hw, hw, 1))
            else:
                # equal piece split keeps every piece's compute above the
                # fp16 LDWEIGHTS time so weight loads stay hidden
                pw = hw // 2 if hw > 512 else hw
                for ri, (base_src, base_dst) in enumerate(((0, 0), (1024, hw))):
                    for off in range(0, hw, pw):
                        w = min(pw, hw - off)
                        ps = spsp.tile([P, 512], f32, tag="ps")
                        for dc in range(8):
                            nc.tensor.matmul(
                                ps[:, :w], gt_sb[:, dc, ts(j, P)],
                                xct_sb[:, dc,
                                       base_src + off:base_src + off + w],
                                start=(dc == 0), stop=(dc == 7))
                        dst = base_dst + off
                        nc.vector.tensor_copy(srow[:, dst:dst + w],
                                              ps[:, :w])
                        segs.append((dst, w, ri if off + w == hw else None))
            mxseg = statp.tile([P, 4], f32, tag="mxseg")
            for si, (dst, w, ri) in enumerate(segs):
                if ri is not None:
                    # boundary chunk of region ri sits at this segment's
                    # tail: apply the additive causal mask before the max
                    chunk = ts(j, P) if ri == 0 else ts(2 * j + 1, P)
                    nc.vector.tensor_add(srow[:, chunk], srow[:, chunk],
                                         madd_sb[:, j, ri * P:(ri + 1) * P])
                nc.vector.reduce_max(mxseg[:, si:si + 1],
                                     srow[:, dst:dst + w], axis=AX)
            nmx = statp.tile([P, 1], f32, tag="nmx")
            nc.vector.reduce_max(nmx[:], mxseg[:, :len(segs)], axis=AX,
                                 negate=True)
            nc.scalar.mul(nmx[:], nmx[:], 1.0 / 32.0)
            seseg = statp.tile([P, 4], f32, tag="seseg")
            attn = earlyp.tile([P, 2048], f16, tag="attn")
            for si, (dst, w, _) in enumerate(segs):
                nc.scalar.activation(attn[:, dst:dst + w],
                                     srow[:, dst:dst + w], Exp,
                                     bias=nmx[:], scale=1.0 / 32.0,
                                     accum_out=seseg[:, si:si + 1])
            sumexp = statp.tile([P, 1], f32, tag="se")
            nc.vector.reduce_sum(sumexp[:], seseg[:, :len(segs)], axis=AX)
            rcp = statp.tile([P, 1], f32, tag="rcp")
            nc.vector.reciprocal(rcp[:], sumexp[:])
            state[j] = (attn, rcp)

        # ---- phase G: G^T = (W_Q W_K^T)^T X_q^T -------------------------
        # HAM warm-up: dependency-free matmuls fill the PE-idle window while
        # the first input chunks stream in, so phase G starts at full clock
        warm = spsp.tile([P, 512], f32, tag="ps", name="warmup")
        for _ in range(WARMUP):
            nc.tensor.matmul(warm[:, 0:P], ident_h[:], ident_h[:])

        with tc.tile_pool(name="wqk", bufs=1) as wqkp, \
             tc.tile_pool(name="pps", bufs=6, space="PSUM") as pps:
            wqk_sb = wqkp.tile([P, 8, 1024], f16)
            # all bulk inputs on the sync queue in first-use order; the
            # scalar engine stays free for the gt copies / exp / out path
            nc.sync.dma_start(wqk_sb[:, 0:4, :], wqk_p[:, 0:4, :])
            nc.sync.dma_start(xct_sb[:, 0:4, 0:512], xct_p[:, 0:4, 0:512])
            nc.sync.dma_start(wqk_sb[:, 4:8, :], wqk_p[:, 4:8, :])
            nc.sync.dma_start(xct_sb[:, 4:8, 0:512], xct_p[:, 4:8, 0:512])
            nc.sync.dma_start(xct_sb[:, :, 512:1024], xct_p[:, :, 512:1024])
            # region-2 head: needed by the pipelined scores_0/1
            nc.sync.dma_start(xct_sb[:, :, 1024:1280], xct_p[:, :, 1024:1280])
            nc.scalar.dma_start(madd_sb[:], madd_r)
            nc.sync.dma_start(xc_sb[:, 0, :], xc_p[:, 0, :])
            nc.sync.dma_start(xc_sb[:, 8, :], xc_p[:, 8, :])
            nc.sync.dma_start(wv_sb[:], wv_p)
            nc.sync.dma_start(xc_sb[:, 1:8, :], xc_p[:, 1:8, :])
            nc.sync.dma_start(xc_sb[:, 9:16, :], xc_p[:, 9:16, :])
            nc.sync.dma_start(xct_sb[:, :, 1280:1536], xct_p[:, :, 1280:1536])
            nc.sync.dma_start(xct_sb[:, :, 1536:2048], xct_p[:, :, 1536:2048])

            # qh-outer: pass 0 needs only the first xct q-half from DRAM;
            # scores_0 slots between the passes so its softmax runs under
            # pass 1.
            for qh in (0, 512):
                for g0 in range(0, 8, 2):
                    psl = {dt_: pps.tile([P, 512], f32, tag="ps",
                                         name=f"psG{dt_}{qh}")
                           for dt_ in (g0, g0 + 1)}
                    for dc in range(8):
                        for dt_ in (g0, g0 + 1):
                            nc.tensor.matmul(
                                psl[dt_][:], wqk_sb[:, dc, ts(dt_, P)],
                                xct_sb[:, dc, qh:qh + 512],
                                start=(dc == 0), stop=(dc == 7))
                    for dt_ in (g0, g0 + 1):
                        nc.scalar.copy(gt_sb[:, dt_, qh:qh + 512],
                                       psl[dt_][:])
                if qh == 0:
                    emit_scores_softmax(0)

        # ---- phase D: attention per q-tile, pipelined one stage deep ----
        with tc.tile_pool(name="work1", bufs=1) as work1, \
             tc.tile_pool(name="work2", bufs=2) as work2, \
             tc.tile_pool(name="trp", bufs=2, space="PSUM") as trp, \
             tc.tile_pool(name="ppp", bufs=2, space="PSUM") as ppp, \
             tc.tile_pool(name="ops", bufs=2, space="PSUM") as opsp:
            for j in range(NJ):
                nk = 2 * j + 2          # 128-wide k-chunks this q-tile
                attn, rcp = state.pop(j)

                attnT = work1.tile([P, 2048], f16, tag="attnT")
                for c in range(nk):
                    tp = trp.tile([P, P], f16, tag="tr")
                    nc.tensor.transpose(tp[:], attn[:, ts(c, P)], ident_h[:])
                    nc.vector.tensor_copy(attnT[:, ts(c, P)], tp[:])

                # A @ X_ctx, dv-half-outer so the half-0 copy overlaps the
                # half-1 chain
                p_sb = work2.tile([P, 1024], f16, tag="p", bufs=1)
                for dh in (0, 512):
                    pp = ppp.tile([P, 512], f32, tag="pp")
                    for c in range(nk):
                        pos = c if c <= j else 8 + (c - j - 1)
                        nc.tensor.matmul(
                            pp[:], attnT[:, ts(c, P)],
                            xc_sb[:, pos, dh:dh + 512],
                            start=(c == 0), stop=(c == nk - 1))
                    nc.vector.tensor_copy(p_sb[:, dh:dh + 512], pp[:])

                # next tile's scores+softmax hide under this tile's tail work
                if j + 1 < NJ:
                    emit_scores_softmax(j + 1)

                pt_sb = work2.tile([P, 1024], f16, tag="pt", bufs=1)
                for dc in range(8):
                    tp = trp.tile([P, P], f16, tag="tr")
                    nc.tensor.transpose(tp[:], p_sb[:, ts(dc, P)], ident_h[:])
                    nc.vector.tensor_copy(pt_sb[:, ts(dc, P)], tp[:])

                # P @ W_V, dv-half-outer; each half normalizes and streams
                # out on its own DGE queue as soon as its chain retires
                out_sb = work2.tile([P, 1024], f32, tag="out")
                for dvh, eng in ((0, nc.sync), (512, nc.scalar)):
                    op = opsp.tile([P, 512], f32, tag="op")
                    for dc in range(8):
                        nc.tensor.matmul(
                            op[:], pt_sb[:, ts(dc, P)],
                            wv_sb[:, dc, dvh:dvh + 512],
                            start=(dc == 0), stop=(dc == 7))
                    nc.scalar.activation(out_sb[:, dvh:dvh + 512], op[:],
                                         Copy, scale=rcp[:])
                    eng.dma_start(out_d[ts(j, P), dvh:dvh + 512],
                                  out_sb[:, dvh:dvh + 512])
        statp.release()
        earlyp.release()
        spsp.release()
        wv_pool.release()

    nc.compile()
    return nc


def _prep_inputs(sequence_repr, W_Q, W_K, W_V, mask):
    """Build the 8 per-core input dicts (host-side slicing/permutation)."""
    wqk = (W_Q @ W_K.T).astype(np.float16)
    wv16 = np.ascontiguousarray(W_V).astype(np.float16)
    in_maps = []
    meta = []
    for c in range(NCORES):
        b, par = divmod(c, 2)
        qblocks = [2 * j + par for j in range(NJ)]
        oblocks = [2 * j + 1 - par for j in range(NJ)]
        posblocks = qblocks + oblocks
        rows_perm = np.concatenate(
            [np.arange(g * P, (g + 1) * P) for g in posblocks])
        qrows = rows_perm[:NJ * P]
        xb = sequence_repr[b]
        xct = np.ascontiguousarray(xb.T[:, rows_perm]).astype(np.float16)
        xc = np.ascontiguousarray(xb[rows_perm]).astype(np.float16)
        madd = np.empty((NJ * P, 2 * P), np.float32)
        for j in range(NJ):
            g = 2 * j + par
            gb = 2 * j + 1 - par
            qr = slice((2 * j + par) * P, (2 * j + par) * P + P)
            madd[j * P:(j + 1) * P, 0:P] = np.where(
                mask[b, qr, g * P:(g + 1) * P], 0.0, MASK_FILL)
            madd[j * P:(j + 1) * P, P:2 * P] = np.where(
                mask[b, qr, gb * P:(gb + 1) * P], 0.0, MASK_FILL)
        in_maps.append({
            "xct": xct, "xc": xc,
            "wqk": wqk,
            "wv": wv16,
            "madd": madd,
        })
        meta.append((b, qrows))
    return in_maps, meta


def run(sequence_repr, W_Q, W_K, W_V, mask, trace=False):
    from concourse.bass_utils import run_bass_kernel_spmd

    if "nc" not in _cache:
        _cache["nc"] = _build_program()
    nc = _cache["nc"]
    in_maps, meta = _prep_inputs(
        np.asarray(sequence_repr, np.float32), np.asarray(W_Q, np.float32),
        np.asarray(W_K, np.float32), np.asarray(W_V, np.float32),
        np.asarray(mask))
    res = run_bass_kernel_spmd(nc, in_maps, core_ids=list(range(NCORES)),
                               trace=trace)
    out = np.empty((B, S, D), np.float32)
    for c in range(NCORES):
        b, qrows = meta[c]
        out[b, qrows] = res.results[c]["out"]
    return out, res


def kernel(**inputs):
    out, _ = run(**inputs)
    return out


# revision 14
# speedup vs baseline: 1.1714x; 1.0175x over previous
"""Causal single-head attention (B=4, S=2048, D=DK=1024) on 8 trn2 NeuronCores.

Sharding: data-parallel over batch x interleaved q-blocks. Core c handles
batch b=c//2, parity p=c%2, owning the 8 q-blocks {2j+p : j in 0..7} (128 rows
each). One uniform SPMD program runs on all 8 cores; per-core differences are
carried entirely by the input data (host-side column permutation + mask tiles).

Math per core (weight-folded to skip full-context K/V projections; W_QK =
W_Q W_K^T is folded on the host):
    G^T = W_QK^T X_q^T                [d, 1024]
    S   = G X_ctx^T   (causal window, compact 2-region layout)
    A   = softmax(S/32 with -1e9 mask pre-scale)
    P   = A X_ctx
    out = P W_V       (then scatter rows back on host)

All matmul operands are fp16 (11-bit mantissa, same class as tf32): the PE
streams fp16 at 1 cycle/row like f32r, but LDWEIGHTS takes ~97 ns instead of
~224 so weight loads hide under the 512-col matmul compute (~216 ns), and
every DMA/SBUF byte count halves vs fp32.

The q-tile loop is software-pipelined one stage deep: scores+softmax of tile
j+1 are emitted between A@X and P@W_V of tile j, so each tile's softmax
latency (vector/scalar chain) hides under the previous tile's tensor work.
scores_0 runs between the two G half-passes. Inputs stream on both hardware
DGE queues (sync + scalar) in first-use order.
"""

import numpy as np

B, S, D = 4, 2048, 1024
P = 128               # partitions
NJ = 8                # q-tiles per core
NCORES = 8
MASK_FILL = -1.0e9
WARMUP = 36           # PE clock-ramp matmuls while first inputs stream in

_cache = {}


def _build_program():
    from contextlib import ExitStack
    import concourse.bass as bass
    import concourse.bacc as bacc
    import concourse.tile as tile
    import concourse.mybir as mybir
    from concourse import masks

    f32 = mybir.dt.float32
    f16 = mybir.dt.float16
    Exp = mybir.ActivationFunctionType.Exp
    Copy = mybir.ActivationFunctionType.Copy
    AX = mybir.AxisListType.X
    ts = bass.ts

    nc = bacc.Bacc("TRN2", target_bir_lowering=False, debug=False,
                   enable_asserts=False)

    xct_d = nc.dram_tensor("xct", [D, S], f16, kind="ExternalInput").ap()
    xc_d = nc.dram_tensor("xc", [S, D], f16, kind="ExternalInput").ap()
    wqk_d = nc.dram_tensor("wqk", [D, D], f16, kind="ExternalInput").ap()
    wv_d = nc.dram_tensor("wv", [D, D], f16, kind="ExternalInput").ap()
    madd_d = nc.dram_tensor("madd", [NJ * P, 2 * P], f32,
                            kind="ExternalInput").ap()
    out_d = nc.dram_tensor("out", [NJ * P, D], f32, kind="ExternalOutput").ap()

    # partition-major views so one dma_start covers many 128-row chunks:
    # dma_start instructions cost ~600ns each on the issuing engine, so
    # bulk inputs go as a few large transfers on the sync engine only
    xct_p = xct_d.rearrange("(c p) k -> p c k", p=P)    # [128, 8, 2048]
    xc_p = xc_d.rearrange("(c p) d -> p c d", p=P)      # [128, 16, 1024]
    wqk_p = wqk_d.rearrange("(c p) n -> p c n", p=P)    # [128, 8, 1024]
    wv_p = wv_d.rearrange("(c p) n -> p c n", p=P)
    madd_r = madd_d.rearrange("(j p) m -> p j m", p=P)  # [128, 8, 256]

    with tile.TileContext(nc) as tc, ExitStack() as es:
        # ---- persistent pools -------------------------------------------
        perm = es.enter_context(tc.tile_pool(name="perm", bufs=1))
        xct_sb = perm.tile([P, 8, S], f16)         # X_ctx^T  32KB/part
        xc_sb = perm.tile([P, 16, D], f16)         # X_ctx (perm rows) 32KB/part
        gt_sb = perm.tile([P, 8, 1024], f16)       # G^T 16KB/part
        madd_sb = perm.tile([P, 8, 2 * P], f32)    # all mask tiles 8KB/part
        ident_h = perm.tile([P, P], f16)

        masks.make_identity(nc, ident_h[:])

        wv_pool = tc.alloc_tile_pool(name="wv", bufs=1, side="right")
        wv_sb = wv_pool.tile([P, 8, 1024], f16)
        # scores psum pool + softmax tiles allocated ahead of G's pools so
        # the pipelined scores_0 doesn't wait on pool release barriers
        spsp = tc.alloc_tile_pool(name="sps", bufs=2, space="PSUM")
        earlyp = tc.alloc_tile_pool(name="early", bufs=2)
        statp = tc.alloc_tile_pool(name="stats", bufs=4)

        x4 = xct_sb[:].rearrange("p d (r c) -> p d r c", r=2)

        # tile processing order: tile 0 (the cheapest) goes LAST so the
        # un-pipelined tail work after the final scores stage is minimal
        JSEQ = [1, 2, 3, 4, 5, 6, 7, 0]

        # per-j softmax state threaded between pipeline stages
        state = {}

        def emit_scores_softmax(j):
            hw = (j + 1) * P        # per-region context width
            srow = earlyp.tile([P, 2048], f32, tag="srow")
            segs = []  # (dst_off, width, mask_region) per segment
            if hw <= 256:
                # both regions in one matmul via a 2-region moving AP
                ps = spsp.tile([P, 512], f32, tag="ps")
                for dc in range(8):
                    nc.tensor.matmul(
                        ps[:, 0:2 * hw], gt_sb[:, dc, ts(j, P)],
                        x4[:, dc, :, 0:hw],
                        start=(dc == 0), stop=(dc == 7))
                nc.vector.tensor_copy(srow[:, 0:2 * hw], ps[:, 0:2 * hw])
                segs.append((0, hw, 0))
                segs.append((hw, hw, 1))
            else:
                # equal piece split keeps every piece's compute above the
                # fp16 LDWEIGHTS time so weight loads stay hidden
                pw = hw // 2 if hw > 512 else hw
                for ri, (base_src, base_dst) in enumerate(((0, 0), (1024, hw))):
                    for off in range(0, hw, pw):
                        w = min(pw, hw - off)
                        ps = spsp.tile([P, 512], f32, tag="ps")
                        for dc in range(8):
                            nc.tensor.matmul(
                                ps[:, :w], gt_sb[:, dc, ts(j, P)],
                                xct_sb[:, dc,
                                       base_src + off:base_src + off + w],
                                start=(dc == 0), stop=(dc == 7))
                        dst = base_dst + off
                        nc.vector.tensor_copy(srow[:, dst:dst + w],
                                              ps[:, :w])
                        segs.append((dst, w, ri if off + w == hw else None))
            mxseg = statp.tile([P, 4], f32, tag="mxseg")
            for si, (dst, w, ri) in enumerate(segs):
                if ri is not None:
                    # boundary chunk of region ri sits at this segment's
                    # tail: apply the additive causal mask before the max
                    chunk = ts(j, P) if ri == 0 else ts(2 * j + 1, P)
                    nc.vector.tensor_add(srow[:, chunk], srow[:, chunk],
                                         madd_sb[:, j, ri * P:(ri + 1) * P])
                nc.vector.reduce_max(mxseg[:, si:si + 1],
                                     srow[:, dst:dst + w], axis=AX)
            nmx = statp.tile([P, 1], f32, tag="nmx")
            nc.vector.reduce_max(nmx[:], mxseg[:, :len(segs)], axis=AX,
                                 negate=True)
            nc.scalar.mul(nmx[:], nmx[:], 1.0 / 32.0)
            seseg = statp.tile([P, 4], f32, tag="seseg")
            attn = earlyp.tile([P, 2048], f16, tag="attn")
            for si, (dst, w, _) in enumerate(segs):
                nc.scalar.activation(attn[:, dst:dst + w],
                                     srow[:, dst:dst + w], Exp,
                                     bias=nmx[:], scale=1.0 / 32.0,
                                     accum_out=seseg[:, si:si + 1])
            sumexp = statp.tile([P, 1], f32, tag="se")
            nc.vector.reduce_sum(sumexp[:], seseg[:, :len(segs)], axis=AX)
            rcp = statp.tile([P, 1], f32, tag="rcp")
            nc.vector.reciprocal(rcp[:], sumexp[:])
            state[j] = (attn, rcp)

        # ---- phase G: G^T = (W_Q W_K^T)^T X_q^T -------------------------
        # HAM warm-up: dependency-free matmuls fill the PE-idle window while
        # the first input chunks stream in, so phase G starts at full clock
        warm = spsp.tile([P, 512], f32, tag="ps", name="warmup")
        for _ in range(WARMUP):
            nc.tensor.matmul(warm[:, 0:P], ident_h[:], ident_h[:])

        with tc.tile_pool(name="wqk", bufs=1) as wqkp, \
             tc.tile_pool(name="pps", bufs=6, space="PSUM") as pps:
            wqk_sb = wqkp.tile([P, 8, 1024], f16)
            # all bulk inputs on the sync queue in first-use order; the
            # scalar engine stays free for the gt copies / exp / out path
            nc.sync.dma_start(wqk_sb[:, 0:2, :], wqk_p[:, 0:2, :])
            nc.sync.dma_start(xct_sb[:, 0:2, 0:512], xct_p[:, 0:2, 0:512])
            nc.sync.dma_start(wqk_sb[:, 2:4, :], wqk_p[:, 2:4, :])
            nc.sync.dma_start(xct_sb[:, 2:4, 0:512], xct_p[:, 2:4, 0:512])
            nc.sync.dma_start(wqk_sb[:, 4:8, :], wqk_p[:, 4:8, :])
            nc.sync.dma_start(xct_sb[:, 4:8, 0:512], xct_p[:, 4:8, 0:512])
            nc.sync.dma_start(xct_sb[:, :, 512:1024], xct_p[:, :, 512:1024])
            # region-2 head: needed by the pipelined scores_0/1
            nc.sync.dma_start(xct_sb[:, :, 1024:1280], xct_p[:, :, 1024:1280])
            nc.scalar.dma_start(madd_sb[:], madd_r)
            nc.sync.dma_start(xc_sb[:, 0, :], xc_p[:, 0, :])
            nc.sync.dma_start(xc_sb[:, 8, :], xc_p[:, 8, :])
            nc.sync.dma_start(wv_sb[:], wv_p)
            nc.sync.dma_start(xc_sb[:, 1:8, :], xc_p[:, 1:8, :])
            nc.sync.dma_start(xc_sb[:, 9:16, :], xc_p[:, 9:16, :])
            nc.sync.dma_start(xct_sb[:, :, 1280:1536], xct_p[:, :, 1280:1536])
            nc.sync.dma_start(xct_sb[:, :, 1536:2048], xct_p[:, :, 1536:2048])

            # qh-outer: pass 0 needs only the first xct q-half from DRAM;
            # scores_0 slots between the passes so its softmax runs under
            # pass 1.
            for qh in (0, 512):
                for g0 in range(0, 8, 2):
                    psl = {dt_: pps.tile([P, 512], f32, tag="ps",
                                         name=f"psG{dt_}{qh}")
                           for dt_ in (g0, g0 + 1)}
                    for dc in range(8):
                        for dt_ in (g0, g0 + 1):
                            nc.tensor.matmul(
                                psl[dt_][:], wqk_sb[:, dc, ts(dt_, P)],
                                xct_sb[:, dc, qh:qh + 512],
                                start=(dc == 0), stop=(dc == 7))
                    for dt_ in (g0, g0 + 1):
                        nc.scalar.copy(gt_sb[:, dt_, qh:qh + 512],
                                       psl[dt_][:])
                if qh == 0:
                    emit_scores_softmax(JSEQ[0])

        # ---- phase D: attention per q-tile, pipelined one stage deep ----
        with tc.tile_pool(name="work1", bufs=1) as work1, \
             tc.tile_pool(name="work2", bufs=2) as work2, \
             tc.tile_pool(name="trp", bufs=2, space="PSUM") as trp, \
             tc.tile_pool(name="ppp", bufs=2, space="PSUM") as ppp, \
             tc.tile_pool(name="ops", bufs=2, space="PSUM") as opsp:
            for idx, j in enumerate(JSEQ):
                nk = 2 * j + 2          # 128-wide k-chunks this q-tile
                attn, rcp = state.pop(j)

                attnT = work1.tile([P, 2048], f16, tag="attnT")
                for c in range(nk):
                    tp = trp.tile([P, P], f16, tag="tr")
                    nc.tensor.transpose(tp[:], attn[:, ts(c, P)], ident_h[:])
                    nc.vector.tensor_copy(attnT[:, ts(c, P)], tp[:])

                # A @ X_ctx, dv-half-outer so the half-0 copy overlaps the
                # half-1 chain
                p_sb = work2.tile([P, 1024], f16, tag="p", bufs=1)
                for dh in (0, 512):
                    pp = ppp.tile([P, 512], f32, tag="pp")
                    for c in range(nk):
                        pos = c if c <= j else 8 + (c - j - 1)
                        nc.tensor.matmul(
                            pp[:], attnT[:, ts(c, P)],
                            xc_sb[:, pos, dh:dh + 512],
                            start=(c == 0), stop=(c == nk - 1))
                    nc.vector.tensor_copy(p_sb[:, dh:dh + 512], pp[:])

                # next tile's scores+softmax hide under this tile's tail work
                if idx + 1 < NJ:
                    emit_scores_softmax(JSEQ[idx + 1])

                pt_sb = work2.tile([P, 1024], f16, tag="pt", bufs=1)
                for dc in range(8):
                    tp = trp.tile([P, P], f16, tag="tr")
                    nc.tensor.transpose(tp[:], p_sb[:, ts(dc, P)], ident_h[:])
                    nc.vector.tensor_copy(pt_sb[:, ts(dc, P)], tp[:])

                # P @ W_V, dv-half-outer; each half normalizes and streams
                # out on its own DGE queue as soon as its chain retires
                out_sb = work2.tile([P, 1024], f32, tag="out")
                for dvh, eng in ((0, nc.sync), (512, nc.scalar)):
                    op = opsp.tile([P, 512], f32, tag="op")
                    for dc in range(8):
                        nc.tensor.matmul(
                            op[:], pt_sb[:, ts(dc, P)],
                            wv_sb[:, dc, dvh:dvh + 512],
                            start=(dc == 0), stop=(dc == 7))
                    nc.scalar.activation(out_sb[:, dvh:dvh + 512], op[:],
                                         Copy, scale=rcp[:])
                    eng.dma_start(out_d[ts(j, P), dvh:dvh + 512],
                                  out_sb[:, dvh:dvh + 512])
        statp.release()
        earlyp.release()
        spsp.release()
        wv_pool.release()

    nc.compile()
    return nc


def _prep_inputs(sequence_repr, W_Q, W_K, W_V, mask):
    """Build the 8 per-core input dicts (host-side slicing/permutation)."""
    wqk = (W_Q @ W_K.T).astype(np.float16)
    wv16 = np.ascontiguousarray(W_V).astype(np.float16)
    in_maps = []
    meta = []
    for c in range(NCORES):
        b, par = divmod(c, 2)
        qblocks = [2 * j + par for j in range(NJ)]
        oblocks = [2 * j + 1 - par for j in range(NJ)]
        posblocks = qblocks + oblocks
        rows_perm = np.concatenate(
            [np.arange(g * P, (g + 1) * P) for g in posblocks])
        qrows = rows_perm[:NJ * P]
        xb = sequence_repr[b]
        xct = np.ascontiguousarray(xb.T[:, rows_perm]).astype(np.float16)
        xc = np.ascontiguousarray(xb[rows_perm]).astype(np.float16)
        madd = np.empty((NJ * P, 2 * P), np.float32)
        for j in range(NJ):
            g = 2 * j + par
            gb = 2 * j + 1 - par
            qr = slice((2 * j + par) * P, (2 * j + par) * P + P)
            madd[j * P:(j + 1) * P, 0:P] = np.where(
                mask[b, qr, g * P:(g + 1) * P], 0.0, MASK_FILL)
            madd[j * P:(j + 1) * P, P:2 * P] = np.where(
                mask[b, qr, gb * P:(gb + 1) * P], 0.0, MASK_FILL)
        in_maps.append({
            "xct": xct, "xc": xc,
            "wqk": wqk,
            "wv": wv16,
            "madd": madd,
        })
        meta.append((b, qrows))
    return in_maps, meta


def run(sequence_repr, W_Q, W_K, W_V, mask, trace=False):
    from concourse.bass_utils import run_bass_kernel_spmd

    if "nc" not in _cache:
        _cache["nc"] = _build_program()
    nc = _cache["nc"]
    in_maps, meta = _prep_inputs(
        np.asarray(sequence_repr, np.float32), np.asarray(W_Q, np.float32),
        np.asarray(W_K, np.float32), np.asarray(W_V, np.float32),
        np.asarray(mask))
    res = run_bass_kernel_spmd(nc, in_maps, core_ids=list(range(NCORES)),
                               trace=trace)
    out = np.empty((B, S, D), np.float32)
    for c in range(NCORES):
        b, qrows = meta[c]
        out[b, qrows] = res.results[c]["out"]
    return out, res


def kernel(**inputs):
    out, _ = run(**inputs)
    return out
